# revision 22
# baseline (speedup 1.0000x reference)
"""CKConv (continuous-kernel causal conv) Trainium2 Bass kernel.

Problem: out[b,o,t] = sum_{ci,k<=t} g[o,ci,k] * x[b,ci,t-k] + bias[o]
with g generated by a tiny SIREN net on relative positions.
Shapes: B=4, CIN=32, COUT=64, T=2048, kernel length K=T+1 (tap 2048 never
contributes for t < T, so only taps 0..2047 are computed).

Sharding: 8 cores = (batch b in 0..3) x (input-channel half h in 0..1).
Each core computes a partial over its 16 input channels for all 64 output
channels; the host adds the two halves and the bias (exact fp32 adds).

Formulation (x-stationary): time tiles of 128. For output tile tt and tap
tile j, the contribution is Xwin(d=tt-j).T @ G(j) where Xwin(d)[r, tloc] =
xpad(128d + tloc + r - 127) is a 128x128 window of the shifted-replicated
input (im2col by a single overlapping-window DMA, partition step +1), and
G(j)[r, o] = g[o, cl, 128j + 127 - r]. The within-tile tap reversal is
obtained for free by feeding the SIREN a block-reversed position vector.
PSUM tile w in {0,1} holds t in [1024w, 1024w+1024) as (tloc, (beta, o));
one matmul per (cl, w, d) covers all valid beta blocks at once (moving
operand with 2 free dims), accumulating over cl and d in PSUM.

Matmul dtype float32r: full PE rate at N>=256 with ~1e-4 relative error.
"""

import numpy as np

B, CIN, COUT, T = 4, 32, 64, 2048
DK = 16
N_CORES = 8
CPC = CIN // 2          # channels per core = 16
XPAD_W = 2560           # 512 left zeros + 2048 data
XC_W = 2432             # im2col window columns
GT_COLS = 16 * 1024     # (jt, cl, o) -> jt*1024 + cl*64 + o


def _build_program(om2: float, dt_conv_name: str):
    import concourse.bass as bass
    import concourse.mybir as mybir
    import concourse.tile as tile
    from concourse import bacc
    from concourse.masks import make_identity

    F32 = mybir.dt.float32
    DTC = getattr(mybir.dt, dt_conv_name)
    AF = mybir.ActivationFunctionType

    nc = bacc.Bacc("TRN2", target_bir_lowering=False, debug=False,
                   num_devices=N_CORES)

    xs = nc.dram_tensor("xs", [CPC, T], F32, kind="ExternalInput")
    posr = nc.dram_tensor("posr", [DK, T], F32, kind="ExternalInput")
    w1v = nc.dram_tensor("w1v", [DK, 1], F32, kind="ExternalInput")
    b1v = nc.dram_tensor("b1v", [DK, 1], F32, kind="ExternalInput")
    w2t = nc.dram_tensor("w2t", [DK, DK], F32, kind="ExternalInput")
    b2v = nc.dram_tensor("b2v", [DK, 1], F32, kind="ExternalInput")
    w3aug = nc.dram_tensor("w3aug", [DK + 1, CPC * COUT], F32,
                           kind="ExternalInput")
    y = nc.dram_tensor("y", [COUT, T], F32, kind="ExternalOutput")
    xpad = nc.dram_tensor("xpad", [CPC, XPAD_W], DTC)

    with tile.TileContext(nc) as tc:
        with tc.tile_pool(name="const", bufs=1) as const, \
             tc.tile_pool(name="sb", bufs=1) as sb, \
             tc.tile_pool(name="drn", bufs=2) as drn, \
             tc.tile_pool(name="gt", bufs=1) as gtp, \
             tc.tile_pool(name="xcp", bufs=3) as xcp, \
             tc.tile_pool(name="psg", bufs=3, space="PSUM") as psg, \
             tc.tile_pool(name="psc", bufs=1, space="PSUM") as psc, \
             tc.tile_pool(name="pst", bufs=2, space="PSUM") as pst:

            # ---------- PE prewarm ----------
            # The HAM clock gate only counts MATMUL streaming as activity; a
            # dense dep-free chain here warms the PE to 2.4 GHz during the
            # SIREN prologue (PE is otherwise idle until ~14us).
            warm_src = const.tile([128, 512], DTC, name="warm")
            nc.vector.memset(warm_src[:].bitcast(F32), 0.0)
            pwarm = pst.tile([128, 512], F32, tag="warm", bufs=1)
            for i in range(28):
                nc.tensor.matmul(pwarm[:], warm_src[:, 0:128], warm_src[:],
                                 start=(i == 0), stop=(i == 27),
                                 skip_group_check=True)

            # ---------- SIREN input DMAs first: they head the critical
            # chain (posr -> h1 -> h2 -> Gt2 -> conv) ----------
            posr_t = const.tile([DK, T], F32)
            nc.sync.dma_start(out=posr_t[:], in_=posr.ap())
            w1v_t = const.tile([DK, 1], F32)
            nc.sync.dma_start(out=w1v_t[:], in_=w1v.ap())
            b1v_t = const.tile([DK, 1], F32)
            nc.sync.dma_start(out=b1v_t[:], in_=b1v.ap())
            w2t_t = const.tile([DK, DK], F32)
            nc.sync.dma_start(out=w2t_t[:], in_=w2t.ap())
            b2v_t = const.tile([DK, 1], F32)
            nc.sync.dma_start(out=b2v_t[:], in_=b2v.ap())
            w3aug_t = const.tile([DK + 1, CPC * COUT], F32)
            nc.sync.dma_start(out=w3aug_t[:], in_=w3aug.ap())
            xt = sb.tile([CPC, T], F32)
            nc.sync.dma_start(out=xt[:], in_=xs.ap())

            # h1 = sin(w1v * posr + b1v), rounded to the conv dtype so the
            # h2 matmuls run single-pass at full PE rate
            h1p = sb.tile([DK, T], F32)
            nc.vector.tensor_scalar(h1p[:], posr_t[:], w1v_t[:], None,
                                    mybir.AluOpType.mult)
            h1 = sb.tile([DK, T], DTC)
            nc.scalar.activation(h1[:], h1p[:], AF.Sin, bias=b1v_t[:])
            w2r = const.tile([DK, DK], DTC)
            nc.vector.tensor_copy(w2r[:], w2t_t[:])

            # x staging on Vector (fast cast); DMAs on the GpSimd queue so
            # they never queue behind the SIREN-input DMAs on Sync
            zp = sb.tile([CPC, XPAD_W], DTC)
            nc.vector.memset(zp[:, 0:512].bitcast(F32), 0.0)
            nc.vector.tensor_copy(zp[:, 512:XPAD_W], xt[:])
            nc.gpsimd.dma_start(out=xpad.ap(), in_=zp[:])

            # h2r = [sin(om2*(w2 @ h1) + om2*b2); ones], written directly in
            # the conv dtype (whole tile pre-set to 1.0 so row DK is ones)
            h2r = sb.tile([DK + 1, T], DTC)
            nc.gpsimd.memset(h2r[:], 1.0)
            for q in range(T // 512):
                ph = psg.tile([DK, 512], F32, tag="g")
                nc.tensor.matmul(ph[:], w2r[:], h1[:, q * 512:(q + 1) * 512],
                                 start=True, stop=True)
                nc.scalar.activation(h2r[0:DK, q * 512:(q + 1) * 512], ph[:],
                                     AF.Sin, bias=b2v_t[:], scale=float(om2))

            w3r = sb.tile([DK + 1, CPC * COUT], DTC)
            nc.vector.tensor_copy(w3r[:], w3aug_t[:])

            # second prewarm chain: bridges the PE lull between the h2
            # matmuls and the first Gt2 matmul (Sin/cast dependencies)
            pwarm2 = pst.tile([128, 512], F32, tag="warm", bufs=1)
            for i in range(20):
                nc.tensor.matmul(pwarm2[:], warm_src[:, 0:128], warm_src[:],
                                 start=(i == 0), stop=(i == 19),
                                 skip_group_check=True)

            # ---------- Gt2, split by input-channel quartet ----------
            # gtq[q][r, jt*256 + (cl%4)*64 + o]; conv for quartet q depends
            # only on gtq[q], so quartet 0 unblocks the conv after 16 copies
            # and the rest of the copies overlap conv matmuls.
            gtq = [gtp.tile([128, 16 * 256], DTC, name=f"gtq{q}")
                   for q in range(4)]
            for jt in range(16):
                for half in range(2):
                    pg = psg.tile([128, 512], F32, tag="g")
                    nc.tensor.matmul(
                        pg[:], h2r[:, jt * 128:(jt + 1) * 128],
                        w3r[:, half * 512:(half + 1) * 512],
                        start=True, stop=True)
                    for qh in range(2):
                        q = 2 * half + qh
                        dst = gtq[q][:, jt * 256:(jt + 1) * 256]
                        srcv = pg[:, qh * 256:(qh + 1) * 256]
                        if qh == 0:
                            nc.vector.tensor_copy(dst, srcv)
                        else:
                            nc.scalar.copy(dst, srcv)
            gtqv = [g[:].rearrange("p (j x) -> p j x", j=16) for g in gtq]

            # ---------- causal conv: accumulate in 2 PSUM banks ----------
            psw = [psc.tile([128, 512], F32, name=f"pw{w}") for w in range(2)]
            for cl in range(CPC):
                xc = xcp.tile([128, XC_W], DTC)
                nc.gpsimd.dma_start(
                    out=xc[:],
                    in_=bass.AP(xpad, cl * XPAD_W + 1, [[1, 128], [1, XC_W]]))
                for w in range(2):
                    dmax = 7 if w == 0 else 15
                    for d in range(dmax + 1):
                        beta0 = max(0, d - 8 * w)
                        nb = 8 - beta0
                        j0 = beta0 + 8 * w - d
                        station = xc[:, 128 * d + 384: 128 * d + 512]
                        q, clq = divmod(cl, 4)
                        moving = gtqv[q][:, j0:j0 + nb, clq * 64:(clq + 1) * 64]
                        nc.tensor.matmul(
                            psw[w][:, beta0 * 64: 512], station, moving,
                            start=(cl == 0 and d == 0),
                            stop=(cl == CPC - 1 and d == dmax),
                            skip_group_check=True)

            # ---------- epilogue: transpose (tloc, (beta,o)) -> (o, t) ----------
            # f32r operands: single-pass transpose at 1.5 cyc/row (vs 4 for
            # fp32) with ~1e-4 rounding, far below the conv dtype error
            F32R = mybir.dt.float32r
            identf = const.tile([128, 128], F32)
            make_identity(nc, identf[:])
            ident = const.tile([128, 128], F32R)
            nc.vector.tensor_copy(ident[:], identf[:])
            out_sb = sb.tile([COUT, T], F32)
            for w in range(2):
                sb_d = drn.tile([128, 512], F32R)
                nc.vector.tensor_copy(sb_d[:], psw[w][:])
                for beta in range(8):
                    pt = pst.tile([COUT, 128], F32R)
                    nc.tensor.transpose(pt[:], sb_d[:, beta * 64:(beta + 1) * 64],
                                        ident[:])
                    dst = out_sb[:, w * 1024 + beta * 128: w * 1024 + (beta + 1) * 128]
                    if beta % 2 == 0:
                        nc.vector.tensor_copy(dst, pt[:])
                    else:
                        nc.scalar.copy(dst, pt[:])
            nc.sync.dma_start(out=y.ap(), in_=out_sb[:])

    nc.compile()
    return nc


def kernel(x, pos_rel, w1, b1, om1, w2, b2, om2, w3, b3, bias,
           dt_conv_name: str = "float32r"):
    from concourse.bass_utils import run_bass_kernel_spmd

    x = np.asarray(x, dtype=np.float32)
    pos_rel = np.asarray(pos_rel, dtype=np.float32)
    w1 = np.asarray(w1, dtype=np.float32)
    b1 = np.asarray(b1, dtype=np.float32)
    om1 = float(np.asarray(om1))
    w2 = np.asarray(w2, dtype=np.float32)
    b2 = np.asarray(b2, dtype=np.float32)
    om2 = float(np.asarray(om2))
    w3 = np.asarray(w3, dtype=np.float32)
    b3 = np.asarray(b3, dtype=np.float32)
    bias = np.asarray(bias, dtype=np.float32)

    # block-reversed positions (within each 128-tap tile), taps 0..2047 only,
    # replicated to DK partitions for the broadcast-free h1 compute
    posr_row = pos_rel[:T].reshape(T // 128, 128)[:, ::-1].reshape(T)
    posr = np.ascontiguousarray(
        np.broadcast_to(posr_row[None, :], (DK, T)), dtype=np.float32)

    w1v = (om1 * w1).reshape(DK, 1).astype(np.float32)
    b1v = (om1 * b1).reshape(DK, 1).astype(np.float32)
    w2t = np.ascontiguousarray(w2.T, dtype=np.float32)
    b2v = b2.reshape(DK, 1).astype(np.float32)  # om2 applied as ACT scale

    nc = _build_program(om2, dt_conv_name)

    # per-core inputs
    in_maps = []
    for core in range(N_CORES):
        b, h = divmod(core, 2)
        ci0 = h * CPC
        # w3aug[d, cl*64 + o] = w3[o*32 + ci0 + cl, d]; row DK = b3 slice
        w3_r = w3.reshape(COUT, CIN, DK)[:, ci0:ci0 + CPC, :]   # (o, cl, d)
        w3a = np.transpose(w3_r, (2, 1, 0)).reshape(DK, CPC * COUT)  # d,(cl,o)
        b3_r = b3.reshape(COUT, CIN)[:, ci0:ci0 + CPC]          # (o, cl)
        b3a = np.transpose(b3_r, (1, 0)).reshape(1, CPC * COUT)  # (cl, o)
        w3aug = np.concatenate([w3a, b3a], axis=0).astype(np.float32)
        in_maps.append({
            "xs": np.ascontiguousarray(x[b, ci0:ci0 + CPC, :]),
            "posr": posr,
            "w1v": w1v, "b1v": b1v, "w2t": w2t, "b2v": b2v,
            "w3aug": np.ascontiguousarray(w3aug),
        })

    res = run_bass_kernel_spmd(nc, in_maps, list(range(N_CORES)))

    out = np.empty((B, COUT, T), dtype=np.float32)
    for b in range(B):
        out[b] = res.results[2 * b]["y"] + res.results[2 * b + 1]["y"]
    out += bias[None, :, None]
    return out


# revision 23
# speedup vs baseline: 1.0117x; 1.0117x over previous
"""CKConv (continuous-kernel causal conv) Trainium2 Bass kernel.

Problem: out[b,o,t] = sum_{ci,k<=t} g[o,ci,k] * x[b,ci,t-k] + bias[o]
with g generated by a tiny SIREN net on relative positions.
Shapes: B=4, CIN=32, COUT=64, T=2048, kernel length K=T+1 (tap 2048 never
contributes for t < T, so only taps 0..2047 are computed).

Sharding: 8 cores = (batch b in 0..3) x (input-channel half h in 0..1).
Each core computes a partial over its 16 input channels for all 64 output
channels; the host adds the two halves and the bias (exact fp32 adds).

Formulation (x-stationary): time tiles of 128. For output tile tt and tap
tile j, the contribution is Xwin(d=tt-j).T @ G(j) where Xwin(d)[r, tloc] =
xpad(128d + tloc + r - 127) is a 128x128 window of the shifted-replicated
input (im2col by a single overlapping-window DMA, partition step +1), and
G(j)[r, o] = g[o, cl, 128j + 127 - r]. The within-tile tap reversal is
obtained for free by feeding the SIREN a block-reversed position vector.
PSUM tile w in {0,1} holds t in [1024w, 1024w+1024) as (tloc, (beta, o));
one matmul per (cl, w, d) covers all valid beta blocks at once (moving
operand with 2 free dims), accumulating over cl and d in PSUM.

Matmul dtype float32r: full PE rate at N>=256 with ~1e-4 relative error.
"""

import numpy as np

B, CIN, COUT, T = 4, 32, 64, 2048
DK = 16
N_CORES = 8
CPC = CIN // 2          # channels per core = 16
XPAD_W = 2560           # 512 left zeros + 2048 data
XC_W = 2432             # im2col window columns
GT_COLS = 16 * 1024     # (jt, cl, o) -> jt*1024 + cl*64 + o


def _build_program(om2: float, dt_conv_name: str):
    import concourse.bass as bass
    import concourse.mybir as mybir
    import concourse.tile as tile
    from concourse import bacc
    from concourse.masks import make_identity

    F32 = mybir.dt.float32
    DTC = getattr(mybir.dt, dt_conv_name)
    AF = mybir.ActivationFunctionType

    nc = bacc.Bacc("TRN2", target_bir_lowering=False, debug=False,
                   num_devices=N_CORES)

    xs = nc.dram_tensor("xs", [CPC, T], F32, kind="ExternalInput")
    posr = nc.dram_tensor("posr", [DK, T], F32, kind="ExternalInput")
    w1v = nc.dram_tensor("w1v", [DK, 1], F32, kind="ExternalInput")
    b1v = nc.dram_tensor("b1v", [DK, 1], F32, kind="ExternalInput")
    w2t = nc.dram_tensor("w2t", [DK, DK], F32, kind="ExternalInput")
    b2v = nc.dram_tensor("b2v", [DK, 1], F32, kind="ExternalInput")
    w3aug = nc.dram_tensor("w3aug", [DK + 1, CPC * COUT], F32,
                           kind="ExternalInput")
    y = nc.dram_tensor("y", [COUT, T], F32, kind="ExternalOutput")
    xpad = nc.dram_tensor("xpad", [CPC, XPAD_W], DTC)

    with tile.TileContext(nc) as tc:
        with tc.tile_pool(name="const", bufs=1) as const, \
             tc.tile_pool(name="sb", bufs=1) as sb, \
             tc.tile_pool(name="drn", bufs=2) as drn, \
             tc.tile_pool(name="gt", bufs=1) as gtp, \
             tc.tile_pool(name="xcp", bufs=3) as xcp, \
             tc.tile_pool(name="psg", bufs=4, space="PSUM") as psg, \
             tc.tile_pool(name="psc", bufs=1, space="PSUM") as psc, \
             tc.tile_pool(name="pst", bufs=2, space="PSUM") as pst:

            # ---------- PE prewarm ----------
            # The HAM clock gate only counts MATMUL streaming as activity; a
            # dense dep-free chain here warms the PE to 2.4 GHz during the
            # SIREN prologue (PE is otherwise idle until ~14us).
            warm_src = const.tile([128, 512], DTC, name="warm")
            nc.vector.memset(warm_src[:].bitcast(F32), 0.0)
            pwarm = pst.tile([128, 512], F32, tag="warm", bufs=1)
            for i in range(28):
                nc.tensor.matmul(pwarm[:], warm_src[:, 0:128], warm_src[:],
                                 start=(i == 0), stop=(i == 27),
                                 skip_group_check=True)

            # ---------- SIREN input DMAs first: they head the critical
            # chain (posr -> h1 -> h2 -> Gt2 -> conv) ----------
            posr_t = const.tile([DK, T], F32)
            nc.sync.dma_start(out=posr_t[:], in_=posr.ap())
            w1v_t = const.tile([DK, 1], F32)
            nc.sync.dma_start(out=w1v_t[:], in_=w1v.ap())
            b1v_t = const.tile([DK, 1], F32)
            nc.sync.dma_start(out=b1v_t[:], in_=b1v.ap())
            w2t_t = const.tile([DK, DK], F32)
            nc.sync.dma_start(out=w2t_t[:], in_=w2t.ap())
            b2v_t = const.tile([DK, 1], F32)
            nc.sync.dma_start(out=b2v_t[:], in_=b2v.ap())
            w3aug_t = const.tile([DK + 1, CPC * COUT], F32)
            nc.sync.dma_start(out=w3aug_t[:], in_=w3aug.ap())
            xt = sb.tile([CPC, T], F32)
            nc.sync.dma_start(out=xt[:], in_=xs.ap())

            # h1 = sin(w1v * posr + b1v), rounded to the conv dtype so the
            # h2 matmuls run single-pass at full PE rate
            h1p = sb.tile([DK, T], F32)
            nc.vector.tensor_scalar(h1p[:], posr_t[:], w1v_t[:], None,
                                    mybir.AluOpType.mult)
            h1 = sb.tile([DK, T], DTC)
            nc.scalar.activation(h1[:], h1p[:], AF.Sin, bias=b1v_t[:])
            w2r = const.tile([DK, DK], DTC)
            nc.vector.tensor_copy(w2r[:], w2t_t[:])

            # x staging on Vector (fast cast); DMAs on the GpSimd queue so
            # they never queue behind the SIREN-input DMAs on Sync
            zp = sb.tile([CPC, XPAD_W], DTC)
            nc.vector.memset(zp[:, 0:512].bitcast(F32), 0.0)
            nc.vector.tensor_copy(zp[:, 512:XPAD_W], xt[:])
            nc.gpsimd.dma_start(out=xpad.ap(), in_=zp[:])

            # h2r = [sin(om2*(w2 @ h1) + om2*b2); ones], written directly in
            # the conv dtype (whole tile pre-set to 1.0 so row DK is ones)
            h2r = sb.tile([DK + 1, T], DTC)
            nc.gpsimd.memset(h2r[:], 1.0)
            for q in range(T // 512):
                ph = psg.tile([DK, 512], F32, tag="g")
                nc.tensor.matmul(ph[:], w2r[:], h1[:, q * 512:(q + 1) * 512],
                                 start=True, stop=True)
                nc.scalar.activation(h2r[0:DK, q * 512:(q + 1) * 512], ph[:],
                                     AF.Sin, bias=b2v_t[:], scale=float(om2))

            w3r = sb.tile([DK + 1, CPC * COUT], DTC)
            nc.vector.tensor_copy(w3r[:], w3aug_t[:])


            # ---------- Gt2, split by input-channel quartet ----------
            # gtq[q][r, jt*256 + (cl%4)*64 + o]; conv for quartet q depends
            # only on gtq[q], so quartet 0 unblocks the conv after 16 copies
            # and the rest of the copies overlap conv matmuls.
            gtq = [gtp.tile([128, 16 * 256], DTC, name=f"gtq{q}")
                   for q in range(4)]
            for jt in range(16):
                for half in range(2):
                    pg = psg.tile([128, 512], F32, tag="g")
                    nc.tensor.matmul(
                        pg[:], h2r[:, jt * 128:(jt + 1) * 128],
                        w3r[:, half * 512:(half + 1) * 512],
                        start=True, stop=True)
                    for qh in range(2):
                        q = 2 * half + qh
                        dst = gtq[q][:, jt * 256:(jt + 1) * 256]
                        srcv = pg[:, qh * 256:(qh + 1) * 256]
                        if qh == 0:
                            nc.vector.tensor_copy(dst, srcv)
                        else:
                            nc.scalar.copy(dst, srcv)
            gtqv = [g[:].rearrange("p (j x) -> p j x", j=16) for g in gtq]

            # ---------- causal conv: accumulate in 2 PSUM banks ----------
            psw = [psc.tile([128, 512], F32, name=f"pw{w}") for w in range(2)]
            for cl in range(CPC):
                xc = xcp.tile([128, XC_W], DTC)
                nc.gpsimd.dma_start(
                    out=xc[:],
                    in_=bass.AP(xpad, cl * XPAD_W + 1, [[1, 128], [1, XC_W]]))
                for w in range(2):
                    dmax = 7 if w == 0 else 15
                    for d in range(dmax + 1):
                        beta0 = max(0, d - 8 * w)
                        nb = 8 - beta0
                        j0 = beta0 + 8 * w - d
                        station = xc[:, 128 * d + 384: 128 * d + 512]
                        q, clq = divmod(cl, 4)
                        moving = gtqv[q][:, j0:j0 + nb, clq * 64:(clq + 1) * 64]
                        nc.tensor.matmul(
                            psw[w][:, beta0 * 64: 512], station, moving,
                            start=(cl == 0 and d == 0),
                            stop=(cl == CPC - 1 and d == dmax),
                            skip_group_check=True)

            # ---------- epilogue: transpose (tloc, (beta,o)) -> (o, t) ----------
            # f32r operands: single-pass transpose at 1.5 cyc/row (vs 4 for
            # fp32) with ~1e-4 rounding, far below the conv dtype error
            F32R = mybir.dt.float32r
            identf = const.tile([128, 128], F32)
            make_identity(nc, identf[:])
            ident = const.tile([128, 128], F32R)
            nc.vector.tensor_copy(ident[:], identf[:])
            out_sb = sb.tile([COUT, T], F32)
            for w in range(2):
                sb_d = drn.tile([128, 512], F32R)
                nc.vector.tensor_copy(sb_d[:], psw[w][:])
                for beta in range(8):
                    pt = pst.tile([COUT, 128], F32R, bufs=1)
                    nc.tensor.transpose(pt[:], sb_d[:, beta * 64:(beta + 1) * 64],
                                        ident[:])
                    dst = out_sb[:, w * 1024 + beta * 128: w * 1024 + (beta + 1) * 128]
                    if beta % 2 == 0:
                        nc.vector.tensor_copy(dst, pt[:])
                    else:
                        nc.scalar.copy(dst, pt[:])
            nc.sync.dma_start(out=y.ap(), in_=out_sb[:])

    nc.compile()
    return nc


def kernel(x, pos_rel, w1, b1, om1, w2, b2, om2, w3, b3, bias,
           dt_conv_name: str = "float32r"):
    from concourse.bass_utils import run_bass_kernel_spmd

    x = np.asarray(x, dtype=np.float32)
    pos_rel = np.asarray(pos_rel, dtype=np.float32)
    w1 = np.asarray(w1, dtype=np.float32)
    b1 = np.asarray(b1, dtype=np.float32)
    om1 = float(np.asarray(om1))
    w2 = np.asarray(w2, dtype=np.float32)
    b2 = np.asarray(b2, dtype=np.float32)
    om2 = float(np.asarray(om2))
    w3 = np.asarray(w3, dtype=np.float32)
    b3 = np.asarray(b3, dtype=np.float32)
    bias = np.asarray(bias, dtype=np.float32)

    # block-reversed positions (within each 128-tap tile), taps 0..2047 only,
    # replicated to DK partitions for the broadcast-free h1 compute
    posr_row = pos_rel[:T].reshape(T // 128, 128)[:, ::-1].reshape(T)
    posr = np.ascontiguousarray(
        np.broadcast_to(posr_row[None, :], (DK, T)), dtype=np.float32)

    w1v = (om1 * w1).reshape(DK, 1).astype(np.float32)
    b1v = (om1 * b1).reshape(DK, 1).astype(np.float32)
    w2t = np.ascontiguousarray(w2.T, dtype=np.float32)
    b2v = b2.reshape(DK, 1).astype(np.float32)  # om2 applied as ACT scale

    nc = _build_program(om2, dt_conv_name)

    # per-core inputs
    in_maps = []
    for core in range(N_CORES):
        b, h = divmod(core, 2)
        ci0 = h * CPC
        # w3aug[d, cl*64 + o] = w3[o*32 + ci0 + cl, d]; row DK = b3 slice
        w3_r = w3.reshape(COUT, CIN, DK)[:, ci0:ci0 + CPC, :]   # (o, cl, d)
        w3a = np.transpose(w3_r, (2, 1, 0)).reshape(DK, CPC * COUT)  # d,(cl,o)
        b3_r = b3.reshape(COUT, CIN)[:, ci0:ci0 + CPC]          # (o, cl)
        b3a = np.transpose(b3_r, (1, 0)).reshape(1, CPC * COUT)  # (cl, o)
        w3aug = np.concatenate([w3a, b3a], axis=0).astype(np.float32)
        in_maps.append({
            "xs": np.ascontiguousarray(x[b, ci0:ci0 + CPC, :]),
            "posr": posr,
            "w1v": w1v, "b1v": b1v, "w2t": w2t, "b2v": b2v,
            "w3aug": np.ascontiguousarray(w3aug),
        })

    res = run_bass_kernel_spmd(nc, in_maps, list(range(N_CORES)))

    out = np.empty((B, COUT, T), dtype=np.float32)
    for b in range(B):
        out[b] = res.results[2 * b]["y"] + res.results[2 * b + 1]["y"]
    out += bias[None, :, None]
    return out


# revision 24
# speedup vs baseline: 1.0140x; 1.0024x over previous
"""CKConv (continuous-kernel causal conv) Trainium2 Bass kernel.

Problem: out[b,o,t] = sum_{ci,k<=t} g[o,ci,k] * x[b,ci,t-k] + bias[o]
with g generated by a tiny SIREN net on relative positions.
Shapes: B=4, CIN=32, COUT=64, T=2048, kernel length K=T+1 (tap 2048 never
contributes for t < T, so only taps 0..2047 are computed).

Sharding: 8 cores = (batch b in 0..3) x (input-channel half h in 0..1).
Each core computes a partial over its 16 input channels for all 64 output
channels; the host adds the two halves and the bias (exact fp32 adds).

Formulation (x-stationary): time tiles of 128. For output tile tt and tap
tile j, the contribution is Xwin(d=tt-j).T @ G(j) where Xwin(d)[r, tloc] =
xpad(128d + tloc + r - 127) is a 128x128 window of the shifted-replicated
input (im2col by a single overlapping-window DMA, partition step +1), and
G(j)[r, o] = g[o, cl, 128j + 127 - r]. The within-tile tap reversal is
obtained for free by feeding the SIREN a block-reversed position vector.
PSUM tile w in {0,1} holds t in [1024w, 1024w+1024) as (tloc, (beta, o));
one matmul per (cl, w, d) covers all valid beta blocks at once (moving
operand with 2 free dims), accumulating over cl and d in PSUM.

Matmul dtype float32r: full PE rate at N>=256 with ~1e-4 relative error.
"""

import numpy as np

B, CIN, COUT, T = 4, 32, 64, 2048
DK = 16
N_CORES = 8
CPC = CIN // 2          # channels per core = 16
XPAD_W = 2560           # 512 left zeros + 2048 data
XC_W = 2432             # im2col window columns
GT_COLS = 16 * 1024     # (jt, cl, o) -> jt*1024 + cl*64 + o


def _build_program(om2: float, dt_conv_name: str):
    import concourse.bass as bass
    import concourse.mybir as mybir
    import concourse.tile as tile
    from concourse import bacc
    from concourse.masks import make_identity

    F32 = mybir.dt.float32
    DTC = getattr(mybir.dt, dt_conv_name)
    AF = mybir.ActivationFunctionType

    nc = bacc.Bacc("TRN2", target_bir_lowering=False, debug=False,
                   num_devices=N_CORES)

    xs = nc.dram_tensor("xs", [CPC, T], F32, kind="ExternalInput")
    posr = nc.dram_tensor("posr", [DK, T], F32, kind="ExternalInput")
    w1v = nc.dram_tensor("w1v", [DK, 1], F32, kind="ExternalInput")
    b1v = nc.dram_tensor("b1v", [DK, 1], F32, kind="ExternalInput")
    w2t = nc.dram_tensor("w2t", [DK, DK], F32, kind="ExternalInput")
    b2v = nc.dram_tensor("b2v", [DK, 1], F32, kind="ExternalInput")
    w3aug = nc.dram_tensor("w3aug", [DK + 1, CPC * COUT], F32,
                           kind="ExternalInput")
    y = nc.dram_tensor("y", [COUT, T], F32, kind="ExternalOutput")
    xpad = nc.dram_tensor("xpad", [CPC, XPAD_W], DTC)

    with tile.TileContext(nc) as tc:
        with tc.tile_pool(name="const", bufs=1) as const, \
             tc.tile_pool(name="sb", bufs=1) as sb, \
             tc.tile_pool(name="drn", bufs=2) as drn, \
             tc.tile_pool(name="gt", bufs=1) as gtp, \
             tc.tile_pool(name="xcp", bufs=3) as xcp, \
             tc.tile_pool(name="psg", bufs=4, space="PSUM") as psg, \
             tc.tile_pool(name="psc", bufs=1, space="PSUM") as psc, \
             tc.tile_pool(name="pst", bufs=2, space="PSUM") as pst:

            # ---------- SIREN input DMAs first: they head the critical
            # chain (posr -> h1 -> h2 -> Gt2 -> conv) ----------
            posr_t = const.tile([DK, T], F32)
            nc.sync.dma_start(out=posr_t[:], in_=posr.ap())
            w1v_t = const.tile([DK, 1], F32)
            nc.sync.dma_start(out=w1v_t[:], in_=w1v.ap())
            b1v_t = const.tile([DK, 1], F32)
            nc.sync.dma_start(out=b1v_t[:], in_=b1v.ap())
            w2t_t = const.tile([DK, DK], F32)
            nc.sync.dma_start(out=w2t_t[:], in_=w2t.ap())
            b2v_t = const.tile([DK, 1], F32)
            nc.sync.dma_start(out=b2v_t[:], in_=b2v.ap())
            w3aug_t = const.tile([DK + 1, CPC * COUT], F32)
            nc.sync.dma_start(out=w3aug_t[:], in_=w3aug.ap())
            xt = sb.tile([CPC, T], F32)
            nc.sync.dma_start(out=xt[:], in_=xs.ap())

            # h1 = sin(w1v * posr + b1v), rounded to the conv dtype so the
            # h2 matmuls run single-pass at full PE rate
            h1p = sb.tile([DK, T], F32)
            nc.vector.tensor_scalar(h1p[:], posr_t[:], w1v_t[:], None,
                                    mybir.AluOpType.mult)
            h1 = sb.tile([DK, T], DTC)
            nc.scalar.activation(h1[:], h1p[:], AF.Sin, bias=b1v_t[:])
            w2r = const.tile([DK, DK], DTC)
            nc.vector.tensor_copy(w2r[:], w2t_t[:])

            # x staging on Vector (fast cast); DMAs on the GpSimd queue so
            # they never queue behind the SIREN-input DMAs on Sync
            zp = sb.tile([CPC, XPAD_W], DTC)
            nc.vector.memset(zp[:, 0:512].bitcast(F32), 0.0)
            nc.vector.tensor_copy(zp[:, 512:XPAD_W], xt[:])
            nc.gpsimd.dma_start(out=xpad.ap(), in_=zp[:])

            # h2r = [sin(om2*(w2 @ h1) + om2*b2); ones], written directly in
            # the conv dtype (whole tile pre-set to 1.0 so row DK is ones)
            h2r = sb.tile([DK + 1, T], DTC)
            nc.gpsimd.memset(h2r[:], 1.0)
            for q in range(T // 512):
                ph = psg.tile([DK, 512], F32, tag="g")
                nc.tensor.matmul(ph[:], w2r[:], h1[:, q * 512:(q + 1) * 512],
                                 start=True, stop=True)
                nc.scalar.activation(h2r[0:DK, q * 512:(q + 1) * 512], ph[:],
                                     AF.Sin, bias=b2v_t[:], scale=float(om2))

            w3r = sb.tile([DK + 1, CPC * COUT], DTC)
            nc.vector.tensor_copy(w3r[:], w3aug_t[:])


            # ---------- Gt2, split by input-channel quartet ----------
            # gtq[q][r, jt*256 + (cl%4)*64 + o]; conv for quartet q depends
            # only on gtq[q], so quartet 0 unblocks the conv after 16 copies
            # and the rest of the copies overlap conv matmuls.
            gtq = [gtp.tile([128, 16 * 256], DTC, name=f"gtq{q}")
                   for q in range(4)]
            for jt in range(16):
                for half in range(2):
                    pg = psg.tile([128, 512], F32, tag="g")
                    nc.tensor.matmul(
                        pg[:], h2r[:, jt * 128:(jt + 1) * 128],
                        w3r[:, half * 512:(half + 1) * 512],
                        start=True, stop=True)
                    for qh in range(2):
                        q = 2 * half + qh
                        dst = gtq[q][:, jt * 256:(jt + 1) * 256]
                        srcv = pg[:, qh * 256:(qh + 1) * 256]
                        if qh == 0:
                            nc.vector.tensor_copy(dst, srcv)
                        else:
                            nc.scalar.copy(dst, srcv)
            gtqv = [g[:].rearrange("p (j x) -> p j x", j=16) for g in gtq]

            # ---------- causal conv: accumulate in 2 PSUM banks ----------
            psw = [psc.tile([128, 512], F32, name=f"pw{w}") for w in range(2)]
            for cl in range(CPC):
                xc = xcp.tile([128, XC_W], DTC)
                nc.gpsimd.dma_start(
                    out=xc[:],
                    in_=bass.AP(xpad, cl * XPAD_W + 1, [[1, 128], [1, XC_W]]))
                for w in range(2):
                    dmax = 7 if w == 0 else 15
                    for d in range(dmax + 1):
                        beta0 = max(0, d - 8 * w)
                        nb = 8 - beta0
                        j0 = beta0 + 8 * w - d
                        station = xc[:, 128 * d + 384: 128 * d + 512]
                        q, clq = divmod(cl, 4)
                        moving = gtqv[q][:, j0:j0 + nb, clq * 64:(clq + 1) * 64]
                        nc.tensor.matmul(
                            psw[w][:, beta0 * 64: 512], station, moving,
                            start=(cl == 0 and d == 0),
                            stop=(cl == CPC - 1 and d == dmax),
                            skip_group_check=True)

            # ---------- epilogue: transpose (tloc, (beta,o)) -> (o, t) ----------
            # f32r operands: single-pass transpose at 1.5 cyc/row (vs 4 for
            # fp32) with ~1e-4 rounding, far below the conv dtype error
            F32R = mybir.dt.float32r
            identf = const.tile([128, 128], F32)
            make_identity(nc, identf[:])
            ident = const.tile([128, 128], F32R)
            nc.vector.tensor_copy(ident[:], identf[:])
            out_sb = sb.tile([COUT, T], F32)
            for w in range(2):
                sb_d = drn.tile([128, 512], F32R)
                nc.vector.tensor_copy(sb_d[:], psw[w][:])
                for beta in range(8):
                    pt = pst.tile([COUT, 128], F32R, bufs=1)
                    nc.tensor.transpose(pt[:], sb_d[:, beta * 64:(beta + 1) * 64],
                                        ident[:])
                    dst = out_sb[:, w * 1024 + beta * 128: w * 1024 + (beta + 1) * 128]
                    if beta % 2 == 0:
                        nc.vector.tensor_copy(dst, pt[:])
                    else:
                        nc.scalar.copy(dst, pt[:])
            nc.sync.dma_start(out=y.ap(), in_=out_sb[:])

    nc.compile()
    return nc


def kernel(x, pos_rel, w1, b1, om1, w2, b2, om2, w3, b3, bias,
           dt_conv_name: str = "float32r"):
    from concourse.bass_utils import run_bass_kernel_spmd

    x = np.asarray(x, dtype=np.float32)
    pos_rel = np.asarray(pos_rel, dtype=np.float32)
    w1 = np.asarray(w1, dtype=np.float32)
    b1 = np.asarray(b1, dtype=np.float32)
    om1 = float(np.asarray(om1))
    w2 = np.asarray(w2, dtype=np.float32)
    b2 = np.asarray(b2, dtype=np.float32)
    om2 = float(np.asarray(om2))
    w3 = np.asarray(w3, dtype=np.float32)
    b3 = np.asarray(b3, dtype=np.float32)
    bias = np.asarray(bias, dtype=np.float32)

    # block-reversed positions (within each 128-tap tile), taps 0..2047 only,
    # replicated to DK partitions for the broadcast-free h1 compute
    posr_row = pos_rel[:T].reshape(T // 128, 128)[:, ::-1].reshape(T)
    posr = np.ascontiguousarray(
        np.broadcast_to(posr_row[None, :], (DK, T)), dtype=np.float32)

    w1v = (om1 * w1).reshape(DK, 1).astype(np.float32)
    b1v = (om1 * b1).reshape(DK, 1).astype(np.float32)
    w2t = np.ascontiguousarray(w2.T, dtype=np.float32)
    b2v = b2.reshape(DK, 1).astype(np.float32)  # om2 applied as ACT scale

    nc = _build_program(om2, dt_conv_name)

    # per-core inputs
    in_maps = []
    for core in range(N_CORES):
        b, h = divmod(core, 2)
        ci0 = h * CPC
        # w3aug[d, cl*64 + o] = w3[o*32 + ci0 + cl, d]; row DK = b3 slice
        w3_r = w3.reshape(COUT, CIN, DK)[:, ci0:ci0 + CPC, :]   # (o, cl, d)
        w3a = np.transpose(w3_r, (2, 1, 0)).reshape(DK, CPC * COUT)  # d,(cl,o)
        b3_r = b3.reshape(COUT, CIN)[:, ci0:ci0 + CPC]          # (o, cl)
        b3a = np.transpose(b3_r, (1, 0)).reshape(1, CPC * COUT)  # (cl, o)
        w3aug = np.concatenate([w3a, b3a], axis=0).astype(np.float32)
        in_maps.append({
            "xs": np.ascontiguousarray(x[b, ci0:ci0 + CPC, :]),
            "posr": posr,
            "w1v": w1v, "b1v": b1v, "w2t": w2t, "b2v": b2v,
            "w3aug": np.ascontiguousarray(w3aug),
        })

    res = run_bass_kernel_spmd(nc, in_maps, list(range(N_CORES)))

    out = np.empty((B, COUT, T), dtype=np.float32)
    for b in range(B):
        out[b] = res.results[2 * b]["y"] + res.results[2 * b + 1]["y"]
    out += bias[None, :, None]
    return out


# revision 25
# speedup vs baseline: 1.0335x; 1.0192x over previous
"""CKConv (continuous-kernel causal conv) Trainium2 Bass kernel.

Problem: out[b,o,t] = sum_{ci,k<=t} g[o,ci,k] * x[b,ci,t-k] + bias[o]
with g generated by a tiny SIREN net on relative positions.
Shapes: B=4, CIN=32, COUT=64, T=2048, kernel length K=T+1 (tap 2048 never
contributes for t < T, so only taps 0..2047 are computed).

Sharding: 8 cores = (batch b in 0..3) x (input-channel half h in 0..1).
Each core computes a partial over its 16 input channels for all 64 output
channels; the host adds the two halves and the bias (exact fp32 adds).

Formulation (x-stationary): time tiles of 128. For output tile tt and tap
tile j, the contribution is Xwin(d=tt-j).T @ G(j) where Xwin(d)[r, tloc] =
xpad(128d + tloc + r - 127) is a 128x128 window of the shifted-replicated
input (im2col by a single overlapping-window DMA, partition step +1), and
G(j)[r, o] = g[o, cl, 128j + 127 - r]. The within-tile tap reversal is
obtained for free by feeding the SIREN a block-reversed position vector.
PSUM tile w in {0,1} holds t in [1024w, 1024w+1024) as (tloc, (beta, o));
one matmul per (cl, w, d) covers all valid beta blocks at once (moving
operand with 2 free dims), accumulating over cl and d in PSUM.

Matmul dtype float32r: full PE rate at N>=256 with ~1e-4 relative error.
"""

import numpy as np

B, CIN, COUT, T = 4, 32, 64, 2048
DK = 16
N_CORES = 8
CPC = CIN // 2          # channels per core = 16
XPAD_W = 2560           # 512 left zeros + 2048 data
XC_W = 2432             # im2col window columns
GT_COLS = 16 * 1024     # (jt, cl, o) -> jt*1024 + cl*64 + o


def _build_program(om2: float, dt_conv_name: str):
    import concourse.bass as bass
    import concourse.mybir as mybir
    import concourse.tile as tile
    from concourse import bacc
    from concourse.masks import make_identity

    F32 = mybir.dt.float32
    DTC = getattr(mybir.dt, dt_conv_name)
    AF = mybir.ActivationFunctionType

    nc = bacc.Bacc("TRN2", target_bir_lowering=False, debug=False,
                   num_devices=N_CORES)

    xs = nc.dram_tensor("xs", [CPC, T], F32, kind="ExternalInput")
    posr = nc.dram_tensor("posr", [DK, T], F32, kind="ExternalInput")
    w1v = nc.dram_tensor("w1v", [DK, 1], F32, kind="ExternalInput")
    b1v = nc.dram_tensor("b1v", [DK, 1], F32, kind="ExternalInput")
    w2t = nc.dram_tensor("w2t", [DK, DK], F32, kind="ExternalInput")
    b2v = nc.dram_tensor("b2v", [DK, 1], F32, kind="ExternalInput")
    w3aug = nc.dram_tensor("w3aug", [DK + 1, CPC * COUT], F32,
                           kind="ExternalInput")
    y = nc.dram_tensor("y", [COUT, T], F32, kind="ExternalOutput")
    xpad = nc.dram_tensor("xpad", [CPC, XPAD_W], DTC)

    with tile.TileContext(nc) as tc:
        with tc.tile_pool(name="const", bufs=1) as const, \
             tc.tile_pool(name="sb", bufs=1) as sb, \
             tc.tile_pool(name="drn", bufs=2) as drn, \
             tc.tile_pool(name="gt", bufs=1) as gtp, \
             tc.tile_pool(name="xcp", bufs=3) as xcp, \
             tc.tile_pool(name="psg", bufs=3, space="PSUM") as psg, \
             tc.tile_pool(name="psc", bufs=1, space="PSUM") as psc, \
             tc.tile_pool(name="pst", bufs=2, space="PSUM") as pst:

            # ---------- SIREN input DMAs first: they head the critical
            # chain (posr -> h1 -> h2 -> Gt2 -> conv) ----------
            posr_t = const.tile([DK, T], F32)
            nc.sync.dma_start(out=posr_t[:], in_=posr.ap())
            w1v_t = const.tile([DK, 1], F32)
            nc.sync.dma_start(out=w1v_t[:], in_=w1v.ap())
            b1v_t = const.tile([DK, 1], F32)
            nc.sync.dma_start(out=b1v_t[:], in_=b1v.ap())
            w2t_t = const.tile([DK, DK], F32)
            nc.sync.dma_start(out=w2t_t[:], in_=w2t.ap())
            b2v_t = const.tile([DK, 1], F32)
            nc.sync.dma_start(out=b2v_t[:], in_=b2v.ap())
            w3aug_t = const.tile([DK + 1, CPC * COUT], F32)
            nc.sync.dma_start(out=w3aug_t[:], in_=w3aug.ap())
            xt = sb.tile([CPC, T], F32)
            nc.sync.dma_start(out=xt[:], in_=xs.ap())

            # h1 = sin(w1v * posr + b1v), rounded to the conv dtype so the
            # h2 matmuls run single-pass at full PE rate
            h1p = sb.tile([DK, T], F32)
            nc.vector.tensor_scalar(h1p[:], posr_t[:], w1v_t[:], None,
                                    mybir.AluOpType.mult)
            h1 = sb.tile([DK, T], DTC)
            nc.scalar.activation(h1[:], h1p[:], AF.Sin, bias=b1v_t[:])
            w2r = const.tile([DK, DK], DTC)
            nc.vector.tensor_copy(w2r[:], w2t_t[:])

            # x staging on Vector (fast cast); DMAs on the GpSimd queue so
            # they never queue behind the SIREN-input DMAs on Sync
            zp = sb.tile([CPC, XPAD_W], DTC)
            nc.vector.memset(zp[:, 0:512].bitcast(F32), 0.0)
            nc.vector.tensor_copy(zp[:, 512:XPAD_W], xt[:])
            nc.gpsimd.dma_start(out=xpad.ap(), in_=zp[:])

            # h2r = [sin(om2*(w2 @ h1) + om2*b2); ones], written directly in
            # the conv dtype (whole tile pre-set to 1.0 so row DK is ones)
            h2r = sb.tile([DK + 1, T], DTC)
            nc.gpsimd.memset(h2r[:], 1.0)
            for q in range(T // 512):
                ph = psg.tile([DK, 512], F32, tag="g")
                nc.tensor.matmul(ph[:], w2r[:], h1[:, q * 512:(q + 1) * 512],
                                 start=True, stop=True)
                nc.scalar.activation(h2r[0:DK, q * 512:(q + 1) * 512], ph[:],
                                     AF.Sin, bias=b2v_t[:], scale=float(om2))

            w3r = sb.tile([DK + 1, CPC * COUT], DTC)
            nc.vector.tensor_copy(w3r[:], w3aug_t[:])


            # ---------- Gt2, split by input-channel quartet ----------
            # gtq[q][r, jt*256 + (cl%4)*64 + o]; conv for quartet q depends
            # only on gtq[q], so quartet 0 unblocks the conv after 16 copies
            # and the rest of the copies overlap conv matmuls.
            gtq = [gtp.tile([128, 16 * 256], DTC, name=f"gtq{q}")
                   for q in range(4)]
            gtqv = [g[:].rearrange("p (j x) -> p j x", j=16) for g in gtq]

            def emit_gt2_half(half):
                for jt in range(16):
                    pg = psg.tile([128, 512], F32, tag="g")
                    nc.tensor.matmul(
                        pg[:], h2r[:, jt * 128:(jt + 1) * 128],
                        w3r[:, half * 512:(half + 1) * 512],
                        start=True, stop=True)
                    for qh in range(2):
                        q = 2 * half + qh
                        dst = gtq[q][:, jt * 256:(jt + 1) * 256]
                        srcv = pg[:, qh * 256:(qh + 1) * 256]
                        if qh == 0:
                            nc.vector.tensor_copy(dst, srcv)
                        else:
                            nc.scalar.copy(dst, srcv)

            # ---------- causal conv: accumulate in 2 PSUM banks ----------
            # Emission interleaves Gt2 halves with conv channel blocks so the
            # conv starts right after the 16 half-0 Gt2 matmuls.
            psw = [psc.tile([128, 512], F32, name=f"pw{w}") for w in range(2)]

            def emit_conv_cl(cl):
                xc = xcp.tile([128, XC_W], DTC)
                nc.gpsimd.dma_start(
                    out=xc[:],
                    in_=bass.AP(xpad, cl * XPAD_W + 1, [[1, 128], [1, XC_W]]))
                for w in range(2):
                    dmax = 7 if w == 0 else 15
                    for d in range(dmax + 1):
                        beta0 = max(0, d - 8 * w)
                        nb = 8 - beta0
                        j0 = beta0 + 8 * w - d
                        station = xc[:, 128 * d + 384: 128 * d + 512]
                        q, clq = divmod(cl, 4)
                        moving = gtqv[q][:, j0:j0 + nb, clq * 64:(clq + 1) * 64]
                        nc.tensor.matmul(
                            psw[w][:, beta0 * 64: 512], station, moving,
                            start=(cl == 0 and d == 0),
                            stop=(cl == CPC - 1 and d == dmax),
                            skip_group_check=True)

            emit_gt2_half(0)
            for cl in range(0, 8):
                emit_conv_cl(cl)
            emit_gt2_half(1)
            for cl in range(8, CPC):
                emit_conv_cl(cl)

            # ---------- epilogue: transpose (tloc, (beta,o)) -> (o, t) ----------
            # f32r operands: single-pass transpose at 1.5 cyc/row (vs 4 for
            # fp32) with ~1e-4 rounding, far below the conv dtype error
            F32R = mybir.dt.float32r
            identf = const.tile([128, 128], F32)
            make_identity(nc, identf[:])
            ident = const.tile([128, 128], F32R)
            nc.vector.tensor_copy(ident[:], identf[:])
            out_sb = sb.tile([COUT, T], F32)
            for w in range(2):
                sb_d = drn.tile([128, 512], F32R)
                nc.vector.tensor_copy(sb_d[:], psw[w][:])
                for beta in range(8):
                    pt = pst.tile([COUT, 128], F32R)
                    nc.tensor.transpose(pt[:], sb_d[:, beta * 64:(beta + 1) * 64],
                                        ident[:])
                    dst = out_sb[:, w * 1024 + beta * 128: w * 1024 + (beta + 1) * 128]
                    if beta % 2 == 0:
                        nc.vector.tensor_copy(dst, pt[:])
                    else:
                        nc.scalar.copy(dst, pt[:])
            nc.sync.dma_start(out=y.ap(), in_=out_sb[:])

    nc.compile()
    return nc


def kernel(x, pos_rel, w1, b1, om1, w2, b2, om2, w3, b3, bias,
           dt_conv_name: str = "float32r"):
    from concourse.bass_utils import run_bass_kernel_spmd

    x = np.asarray(x, dtype=np.float32)
    pos_rel = np.asarray(pos_rel, dtype=np.float32)
    w1 = np.asarray(w1, dtype=np.float32)
    b1 = np.asarray(b1, dtype=np.float32)
    om1 = float(np.asarray(om1))
    w2 = np.asarray(w2, dtype=np.float32)
    b2 = np.asarray(b2, dtype=np.float32)
    om2 = float(np.asarray(om2))
    w3 = np.asarray(w3, dtype=np.float32)
    b3 = np.asarray(b3, dtype=np.float32)
    bias = np.asarray(bias, dtype=np.float32)

    # block-reversed positions (within each 128-tap tile), taps 0..2047 only,
    # replicated to DK partitions for the broadcast-free h1 compute
    posr_row = pos_rel[:T].reshape(T // 128, 128)[:, ::-1].reshape(T)
    posr = np.ascontiguousarray(
        np.broadcast_to(posr_row[None, :], (DK, T)), dtype=np.float32)

    w1v = (om1 * w1).reshape(DK, 1).astype(np.float32)
    b1v = (om1 * b1).reshape(DK, 1).astype(np.float32)
    w2t = np.ascontiguousarray(w2.T, dtype=np.float32)
    b2v = b2.reshape(DK, 1).astype(np.float32)  # om2 applied as ACT scale

    nc = _build_program(om2, dt_conv_name)

    # per-core inputs
    in_maps = []
    for core in range(N_CORES):
        b, h = divmod(core, 2)
        ci0 = h * CPC
        # w3aug[d, cl*64 + o] = w3[o*32 + ci0 + cl, d]; row DK = b3 slice
        w3_r = w3.reshape(COUT, CIN, DK)[:, ci0:ci0 + CPC, :]   # (o, cl, d)
        w3a = np.transpose(w3_r, (2, 1, 0)).reshape(DK, CPC * COUT)  # d,(cl,o)
        b3_r = b3.reshape(COUT, CIN)[:, ci0:ci0 + CPC]          # (o, cl)
        b3a = np.transpose(b3_r, (1, 0)).reshape(1, CPC * COUT)  # (cl, o)
        w3aug = np.concatenate([w3a, b3a], axis=0).astype(np.float32)
        in_maps.append({
            "xs": np.ascontiguousarray(x[b, ci0:ci0 + CPC, :]),
            "posr": posr,
            "w1v": w1v, "b1v": b1v, "w2t": w2t, "b2v": b2v,
            "w3aug": np.ascontiguousarray(w3aug),
        })

    res = run_bass_kernel_spmd(nc, in_maps, list(range(N_CORES)))

    out = np.empty((B, COUT, T), dtype=np.float32)
    for b in range(B):
        out[b] = res.results[2 * b]["y"] + res.results[2 * b + 1]["y"]
    out += bias[None, :, None]
    return out


# revision 26
# speedup vs baseline: 1.1058x; 1.0700x over previous
"""CKConv (continuous-kernel causal conv) Trainium2 Bass kernel.

Problem: out[b,o,t] = sum_{ci,k<=t} g[o,ci,k] * x[b,ci,t-k] + bias[o]
with g generated by a tiny SIREN net on relative positions.
Shapes: B=4, CIN=32, COUT=64, T=2048, kernel length K=T+1 (tap 2048 never
contributes for t < T, so only taps 0..2047 are computed).

Sharding: 8 cores = (batch b in 0..3) x (input-channel half h in 0..1).
Each core computes a partial over its 16 input channels for all 64 output
channels; the host adds the two halves and the bias (exact fp32 adds).

Formulation (x-stationary): time tiles of 128. For output tile tt and tap
tile j, the contribution is Xwin(d=tt-j).T @ G(j) where Xwin(d)[r, tloc] =
xpad(128d + tloc + r - 127) is a 128x128 window of the shifted-replicated
input (im2col by a single overlapping-window DMA, partition step +1), and
G(j)[r, o] = g[o, cl, 128j + 127 - r]. The within-tile tap reversal is
obtained for free by feeding the SIREN a block-reversed position vector.
PSUM tile w in {0,1} holds t in [1024w, 1024w+1024) as (tloc, (beta, o));
one matmul per (cl, w, d) covers all valid beta blocks at once (moving
operand with 2 free dims), accumulating over cl and d in PSUM.

Matmul dtype float32r: full PE rate at N>=256 with ~1e-4 relative error.
"""

import numpy as np

B, CIN, COUT, T = 4, 32, 64, 2048
DK = 16
N_CORES = 8
CPC = CIN // 2          # channels per core = 16
XPAD_W = 2560           # 512 left zeros + 2048 data
XC_W = 2432             # im2col window columns
GT_COLS = 16 * 1024     # (jt, cl, o) -> jt*1024 + cl*64 + o


def _build_program(om2: float, dt_conv_name: str):
    import concourse.bass as bass
    import concourse.mybir as mybir
    import concourse.tile as tile
    from concourse import bacc
    from concourse.masks import make_identity

    F32 = mybir.dt.float32
    DTC = getattr(mybir.dt, dt_conv_name)
    AF = mybir.ActivationFunctionType

    nc = bacc.Bacc("TRN2", target_bir_lowering=False, debug=False,
                   num_devices=N_CORES)

    xs = nc.dram_tensor("xs", [CPC, T], F32, kind="ExternalInput")
    posr = nc.dram_tensor("posr", [DK, T], F32, kind="ExternalInput")
    w1v = nc.dram_tensor("w1v", [DK, 1], F32, kind="ExternalInput")
    b1v = nc.dram_tensor("b1v", [DK, 1], F32, kind="ExternalInput")
    w2t = nc.dram_tensor("w2t", [DK, DK], F32, kind="ExternalInput")
    b2v = nc.dram_tensor("b2v", [DK, 1], F32, kind="ExternalInput")
    w3aug = nc.dram_tensor("w3aug", [DK + 1, CPC * COUT], F32,
                           kind="ExternalInput")
    y = nc.dram_tensor("y", [COUT, T], F32, kind="ExternalOutput")
    xpad = nc.dram_tensor("xpad", [CPC, XPAD_W], DTC)

    with tile.TileContext(nc) as tc:
        with tc.tile_pool(name="const", bufs=1) as const, \
             tc.tile_pool(name="sb", bufs=1) as sb, \
             tc.tile_pool(name="drn", bufs=2) as drn, \
             tc.tile_pool(name="gt", bufs=1) as gtp, \
             tc.tile_pool(name="xcp", bufs=3) as xcp, \
             tc.tile_pool(name="psg", bufs=4, space="PSUM") as psg, \
             tc.tile_pool(name="psc", bufs=1, space="PSUM") as psc, \
             tc.tile_pool(name="pst", bufs=2, space="PSUM") as pst:

            # ---------- SIREN input DMAs first: they head the critical
            # chain (posr -> h1 -> h2 -> Gt2 -> conv) ----------
            posr_t = const.tile([DK, T], F32)
            nc.sync.dma_start(out=posr_t[:], in_=posr.ap())
            w1v_t = const.tile([DK, 1], F32)
            nc.sync.dma_start(out=w1v_t[:], in_=w1v.ap())
            b1v_t = const.tile([DK, 1], F32)
            nc.sync.dma_start(out=b1v_t[:], in_=b1v.ap())
            w2t_t = const.tile([DK, DK], F32)
            nc.sync.dma_start(out=w2t_t[:], in_=w2t.ap())
            b2v_t = const.tile([DK, 1], F32)
            nc.sync.dma_start(out=b2v_t[:], in_=b2v.ap())
            w3aug_t = const.tile([DK + 1, CPC * COUT], F32)
            nc.sync.dma_start(out=w3aug_t[:], in_=w3aug.ap())
            xt = sb.tile([CPC, T], F32)
            nc.sync.dma_start(out=xt[:], in_=xs.ap())

            # h1 = sin(w1v * posr + b1v) in one ACT op (per-partition scale),
            # rounded to the conv dtype so h2 matmuls run single-pass
            h1 = sb.tile([DK, T], DTC)
            nc.scalar.activation(h1[:], posr_t[:], AF.Sin, bias=b1v_t[:],
                                 scale=w1v_t[:])
            w2r = const.tile([DK, DK], DTC)
            nc.vector.tensor_copy(w2r[:], w2t_t[:])

            # x staging on Vector (fast cast); DMAs on the GpSimd queue so
            # they never queue behind the SIREN-input DMAs on Sync
            zp = sb.tile([CPC, XPAD_W], DTC)
            nc.vector.memset(zp[:, 0:512].bitcast(F32), 0.0)
            nc.vector.tensor_copy(zp[:, 512:XPAD_W], xt[:])
            nc.gpsimd.dma_start(out=xpad.ap(), in_=zp[:])

            # h2r = [sin(om2*(w2 @ h1) + om2*b2); ones], written directly in
            # the conv dtype (whole tile pre-set to 1.0 so row DK is ones)
            h2r = sb.tile([DK + 1, T], DTC)
            nc.gpsimd.memset(h2r[:], 1.0)
            for q in range(T // 512):
                ph = psg.tile([DK, 512], F32, tag="g")
                nc.tensor.matmul(ph[:], w2r[:], h1[:, q * 512:(q + 1) * 512],
                                 start=True, stop=True)
                nc.scalar.activation(h2r[0:DK, q * 512:(q + 1) * 512], ph[:],
                                     AF.Sin, bias=b2v_t[:], scale=float(om2))

            w3r = sb.tile([DK + 1, CPC * COUT], DTC)
            nc.vector.tensor_copy(w3r[:], w3aug_t[:])


            # ---------- Gt2, split by input-channel quartet ----------
            # gtq[q][r, jt*256 + (cl%4)*64 + o]; conv for quartet q depends
            # only on gtq[q], so quartet 0 unblocks the conv after 16 copies
            # and the rest of the copies overlap conv matmuls.
            gtq = [gtp.tile([128, 16 * 256], DTC, name=f"gtq{q}")
                   for q in range(4)]
            gtqv = [g[:].rearrange("p (j x) -> p j x", j=16) for g in gtq]

            def emit_gt2_half(half, jts=None):
                for jt in (range(16) if jts is None else jts):
                    pg = psg.tile([128, 512], F32, tag="g")
                    nc.tensor.matmul(
                        pg[:], h2r[:, jt * 128:(jt + 1) * 128],
                        w3r[:, half * 512:(half + 1) * 512],
                        start=True, stop=True)
                    for qh in range(2):
                        q = 2 * half + qh
                        dst = gtq[q][:, jt * 256:(jt + 1) * 256]
                        srcv = pg[:, qh * 256:(qh + 1) * 256]
                        if qh == 0:
                            nc.vector.tensor_copy(dst, srcv)
                        else:
                            nc.scalar.copy(dst, srcv)

            # ---------- causal conv: accumulate in 2 PSUM banks ----------
            # Emission interleaves Gt2 halves with conv channel blocks so the
            # conv starts right after the 16 half-0 Gt2 matmuls.
            psw = [psc.tile([128, 512], F32, name=f"pw{w}") for w in range(2)]

            def emit_conv_cl(cl):
                xc = xcp.tile([128, XC_W], DTC)
                nc.gpsimd.dma_start(
                    out=xc[:],
                    in_=bass.AP(xpad, cl * XPAD_W + 1, [[1, 128], [1, XC_W]]))
                for w in range(2):
                    dmax = 7 if w == 0 else 15
                    for d in range(dmax + 1):
                        beta0 = max(0, d - 8 * w)
                        nb = 8 - beta0
                        j0 = beta0 + 8 * w - d
                        station = xc[:, 128 * d + 384: 128 * d + 512]
                        q, clq = divmod(cl, 4)
                        moving = gtqv[q][:, j0:j0 + nb, clq * 64:(clq + 1) * 64]
                        nc.tensor.matmul(
                            psw[w][:, beta0 * 64: 512], station, moving,
                            start=(cl == 0 and d == 0),
                            stop=(cl == CPC - 1 and d == dmax),
                            skip_group_check=True)

            emit_gt2_half(0)
            for cl in range(0, 4):
                emit_conv_cl(cl)
            for cl in range(4, 8):
                # spread the half-1 Gt2 matmuls between conv blocks to keep
                # the PE duty cycle high (a contiguous block re-throttles HAM)
                emit_gt2_half(1, jts=range(4 * (cl - 4), 4 * (cl - 3)))
                emit_conv_cl(cl)
            for cl in range(8, CPC):
                emit_conv_cl(cl)

            # ---------- epilogue: transpose (tloc, (beta,o)) -> (o, t) ----------
            # f32r operands: single-pass transpose at 1.5 cyc/row (vs 4 for
            # fp32) with ~1e-4 rounding, far below the conv dtype error
            F32R = mybir.dt.float32r
            identf = const.tile([128, 128], F32)
            make_identity(nc, identf[:])
            ident = const.tile([128, 128], F32R)
            nc.vector.tensor_copy(ident[:], identf[:])
            out_sb = sb.tile([COUT, T], F32)
            for w in range(2):
                sb_d = drn.tile([128, 512], F32R)
                nc.vector.tensor_copy(sb_d[:], psw[w][:])
                for beta in range(8):
                    pt = pst.tile([COUT, 128], F32R)
                    nc.tensor.transpose(pt[:], sb_d[:, beta * 64:(beta + 1) * 64],
                                        ident[:])
                    dst = out_sb[:, w * 1024 + beta * 128: w * 1024 + (beta + 1) * 128]
                    if beta % 2 == 0:
                        nc.vector.tensor_copy(dst, pt[:])
                    else:
                        nc.scalar.copy(dst, pt[:])
            nc.sync.dma_start(out=y.ap(), in_=out_sb[:])

    nc.compile()
    return nc


def kernel(x, pos_rel, w1, b1, om1, w2, b2, om2, w3, b3, bias,
           dt_conv_name: str = "float32r"):
    from concourse.bass_utils import run_bass_kernel_spmd

    x = np.asarray(x, dtype=np.float32)
    pos_rel = np.asarray(pos_rel, dtype=np.float32)
    w1 = np.asarray(w1, dtype=np.float32)
    b1 = np.asarray(b1, dtype=np.float32)
    om1 = float(np.asarray(om1))
    w2 = np.asarray(w2, dtype=np.float32)
    b2 = np.asarray(b2, dtype=np.float32)
    om2 = float(np.asarray(om2))
    w3 = np.asarray(w3, dtype=np.float32)
    b3 = np.asarray(b3, dtype=np.float32)
    bias = np.asarray(bias, dtype=np.float32)

    # block-reversed positions (within each 128-tap tile), taps 0..2047 only,
    # replicated to DK partitions for the broadcast-free h1 compute
    posr_row = pos_rel[:T].reshape(T // 128, 128)[:, ::-1].reshape(T)
    posr = np.ascontiguousarray(
        np.broadcast_to(posr_row[None, :], (DK, T)), dtype=np.float32)

    w1v = (om1 * w1).reshape(DK, 1).astype(np.float32)
    b1v = (om1 * b1).reshape(DK, 1).astype(np.float32)
    w2t = np.ascontiguousarray(w2.T, dtype=np.float32)
    b2v = b2.reshape(DK, 1).astype(np.float32)  # om2 applied as ACT scale

    nc = _build_program(om2, dt_conv_name)

    # per-core inputs
    in_maps = []
    for core in range(N_CORES):
        b, h = divmod(core, 2)
        ci0 = h * CPC
        # w3aug[d, cl*64 + o] = w3[o*32 + ci0 + cl, d]; row DK = b3 slice
        w3_r = w3.reshape(COUT, CIN, DK)[:, ci0:ci0 + CPC, :]   # (o, cl, d)
        w3a = np.transpose(w3_r, (2, 1, 0)).reshape(DK, CPC * COUT)  # d,(cl,o)
        b3_r = b3.reshape(COUT, CIN)[:, ci0:ci0 + CPC]          # (o, cl)
        b3a = np.transpose(b3_r, (1, 0)).reshape(1, CPC * COUT)  # (cl, o)
        w3aug = np.concatenate([w3a, b3a], axis=0).astype(np.float32)
        in_maps.append({
            "xs": np.ascontiguousarray(x[b, ci0:ci0 + CPC, :]),
            "posr": posr,
            "w1v": w1v, "b1v": b1v, "w2t": w2t, "b2v": b2v,
            "w3aug": np.ascontiguousarray(w3aug),
        })

    res = run_bass_kernel_spmd(nc, in_maps, list(range(N_CORES)))

    out = np.empty((B, COUT, T), dtype=np.float32)
    for b in range(B):
        out[b] = res.results[2 * b]["y"] + res.results[2 * b + 1]["y"]
    out += bias[None, :, None]
    return out


# revision 28
# speedup vs baseline: 1.1142x; 1.0076x over previous
"""CKConv (continuous-kernel causal conv) Trainium2 Bass kernel.

Problem: out[b,o,t] = sum_{ci,k<=t} g[o,ci,k] * x[b,ci,t-k] + bias[o]
with g generated by a tiny SIREN net on relative positions.
Shapes: B=4, CIN=32, COUT=64, T=2048, kernel length K=T+1 (tap 2048 never
contributes for t < T, so only taps 0..2047 are computed).

Sharding: 8 cores = (batch b in 0..3) x (input-channel half h in 0..1).
Each core computes a partial over its 16 input channels for all 64 output
channels; the host adds the two halves and the bias (exact fp32 adds).

Formulation (x-stationary): time tiles of 128. For output tile tt and tap
tile j, the contribution is Xwin(d=tt-j).T @ G(j) where Xwin(d)[r, tloc] =
xpad(128d + tloc + r - 127) is a 128x128 window of the shifted-replicated
input (im2col by a single overlapping-window DMA, partition step +1), and
G(j)[r, o] = g[o, cl, 128j + 127 - r]. The within-tile tap reversal is
obtained for free by feeding the SIREN a block-reversed position vector.
PSUM tile w in {0,1} holds t in [1024w, 1024w+1024) as (tloc, (beta, o));
one matmul per (cl, w, d) covers all valid beta blocks at once (moving
operand with 2 free dims), accumulating over cl and d in PSUM.

Matmul dtype float32r: full PE rate at N>=256 with ~1e-4 relative error.
"""

import numpy as np

B, CIN, COUT, T = 4, 32, 64, 2048
DK = 16
N_CORES = 8
CPC = CIN // 2          # channels per core = 16
XPAD_W = 2560           # 512 left zeros + 2048 data
XC_W = 2432             # im2col window columns
GT_COLS = 16 * 1024     # (jt, cl, o) -> jt*1024 + cl*64 + o


def _build_program(om2: float, dt_conv_name: str):
    import concourse.bass as bass
    import concourse.mybir as mybir
    import concourse.tile as tile
    from concourse import bacc
    from concourse.masks import make_identity

    F32 = mybir.dt.float32
    DTC = getattr(mybir.dt, dt_conv_name)
    AF = mybir.ActivationFunctionType

    nc = bacc.Bacc("TRN2", target_bir_lowering=False, debug=False,
                   num_devices=N_CORES)

    xs = nc.dram_tensor("xs", [CPC, T], F32, kind="ExternalInput")
    posr = nc.dram_tensor("posr", [DK, T], F32, kind="ExternalInput")
    w1v = nc.dram_tensor("w1v", [DK, 1], F32, kind="ExternalInput")
    b1v = nc.dram_tensor("b1v", [DK, 1], F32, kind="ExternalInput")
    w2t = nc.dram_tensor("w2t", [DK, DK], F32, kind="ExternalInput")
    b2v = nc.dram_tensor("b2v", [DK, 1], F32, kind="ExternalInput")
    w3aug = nc.dram_tensor("w3aug", [DK + 1, CPC * COUT], F32,
                           kind="ExternalInput")
    y = nc.dram_tensor("y", [COUT, T], F32, kind="ExternalOutput")
    xpad = nc.dram_tensor("xpad", [CPC, XPAD_W], DTC)

    with tile.TileContext(nc) as tc:
        with tc.tile_pool(name="const", bufs=1) as const, \
             tc.tile_pool(name="sb", bufs=1) as sb, \
             tc.tile_pool(name="drn", bufs=2) as drn, \
             tc.tile_pool(name="gt", bufs=1) as gtp, \
             tc.tile_pool(name="xcp", bufs=3) as xcp, \
             tc.tile_pool(name="psg", bufs=4, space="PSUM") as psg, \
             tc.tile_pool(name="psc", bufs=1, space="PSUM") as psc, \
             tc.tile_pool(name="pst", bufs=2, space="PSUM") as pst:

            # ---------- PE prewarm + ACT Sin-table preload ----------
            # ~20 dummy matmuls warm the PE clock gate (HAM) during the DMA/
            # SIREN wait so the matmul stream starts at 2.4 GHz; a dummy Sin
            # loads the ACT table off the h1 critical path.
            warm_src = const.tile([128, 512], DTC, name="warm")
            nc.vector.memset(warm_src[:].bitcast(F32), 0.0)
            pwarm = pst.tile([128, 512], F32, tag="pt")
            for i in range(20):
                nc.tensor.matmul(pwarm[:], warm_src[:, 0:128], warm_src[:],
                                 start=(i == 0), stop=(i == 19),
                                 skip_group_check=True)
            sintab = const.tile([DK, 4], F32, name="sintab")
            nc.scalar.activation(sintab[:], warm_src[0:DK, 0:4], AF.Sin)

            # ---------- SIREN input DMAs first: they head the critical
            # chain (posr -> h1 -> h2 -> Gt2 -> conv) ----------
            posr_t = const.tile([DK, T], F32)
            nc.sync.dma_start(out=posr_t[:], in_=posr.ap())
            w1v_t = const.tile([DK, 1], F32)
            nc.sync.dma_start(out=w1v_t[:], in_=w1v.ap())
            b1v_t = const.tile([DK, 1], F32)
            nc.sync.dma_start(out=b1v_t[:], in_=b1v.ap())
            w2t_t = const.tile([DK, DK], F32)
            nc.sync.dma_start(out=w2t_t[:], in_=w2t.ap())
            b2v_t = const.tile([DK, 1], F32)
            nc.sync.dma_start(out=b2v_t[:], in_=b2v.ap())
            w3aug_t = const.tile([DK + 1, CPC * COUT], F32)
            nc.sync.dma_start(out=w3aug_t[:], in_=w3aug.ap())
            xt = sb.tile([CPC, T], F32)
            nc.sync.dma_start(out=xt[:], in_=xs.ap())

            # h1 = sin(w1v * posr + b1v) in one ACT op (per-partition scale),
            # rounded to the conv dtype so h2 matmuls run single-pass
            h1 = sb.tile([DK, T], DTC)
            nc.scalar.activation(h1[:], posr_t[:], AF.Sin, bias=b1v_t[:],
                                 scale=w1v_t[:])
            w2r = const.tile([DK, DK], DTC)
            nc.vector.tensor_copy(w2r[:], w2t_t[:])

            # x staging on Vector (fast cast); DMAs on the GpSimd queue so
            # they never queue behind the SIREN-input DMAs on Sync
            zp = sb.tile([CPC, XPAD_W], DTC)
            nc.vector.memset(zp[:, 0:512].bitcast(F32), 0.0)
            nc.vector.tensor_copy(zp[:, 512:XPAD_W], xt[:])
            nc.gpsimd.dma_start(out=xpad.ap(), in_=zp[:])

            # h2r = [sin(om2*(w2 @ h1) + om2*b2); ones], written directly in
            # the conv dtype (whole tile pre-set to 1.0 so row DK is ones)
            h2r = sb.tile([DK + 1, T], DTC)
            nc.gpsimd.memset(h2r[:], 1.0)
            for q in range(T // 512):
                ph = psg.tile([DK, 512], F32, tag="g")
                nc.tensor.matmul(ph[:], w2r[:], h1[:, q * 512:(q + 1) * 512],
                                 start=True, stop=True)
                nc.scalar.activation(h2r[0:DK, q * 512:(q + 1) * 512], ph[:],
                                     AF.Sin, bias=b2v_t[:], scale=float(om2))

            w3r = sb.tile([DK + 1, CPC * COUT], DTC)
            nc.vector.tensor_copy(w3r[:], w3aug_t[:])


            # ---------- Gt2, split by input-channel quartet ----------
            # gtq[q][r, jt*256 + (cl%4)*64 + o]; conv for quartet q depends
            # only on gtq[q], so quartet 0 unblocks the conv after 16 copies
            # and the rest of the copies overlap conv matmuls.
            gtq = [gtp.tile([128, 16 * 256], DTC, name=f"gtq{q}")
                   for q in range(4)]
            gtqv = [g[:].rearrange("p (j x) -> p j x", j=16) for g in gtq]

            def emit_gt2_half(half, jts=None):
                for jt in (range(16) if jts is None else jts):
                    pg = psg.tile([128, 512], F32, tag="g")
                    nc.tensor.matmul(
                        pg[:], h2r[:, jt * 128:(jt + 1) * 128],
                        w3r[:, half * 512:(half + 1) * 512],
                        start=True, stop=True)
                    for qh in range(2):
                        q = 2 * half + qh
                        dst = gtq[q][:, jt * 256:(jt + 1) * 256]
                        srcv = pg[:, qh * 256:(qh + 1) * 256]
                        if qh == 0:
                            nc.vector.tensor_copy(dst, srcv)
                        else:
                            nc.scalar.copy(dst, srcv)

            # ---------- causal conv: accumulate in 2 PSUM banks ----------
            # Emission interleaves Gt2 halves with conv channel blocks so the
            # conv starts right after the 16 half-0 Gt2 matmuls.
            psw = [psc.tile([128, 512], F32, name=f"pw{w}") for w in range(2)]

            def emit_conv_cl(cl):
                xc = xcp.tile([128, XC_W], DTC)
                nc.gpsimd.dma_start(
                    out=xc[:],
                    in_=bass.AP(xpad, cl * XPAD_W + 1, [[1, 128], [1, XC_W]]))
                for w in range(2):
                    dmax = 7 if w == 0 else 15
                    for d in range(dmax + 1):
                        beta0 = max(0, d - 8 * w)
                        nb = 8 - beta0
                        j0 = beta0 + 8 * w - d
                        station = xc[:, 128 * d + 384: 128 * d + 512]
                        q, clq = divmod(cl, 4)
                        moving = gtqv[q][:, j0:j0 + nb, clq * 64:(clq + 1) * 64]
                        nc.tensor.matmul(
                            psw[w][:, beta0 * 64: 512], station, moving,
                            start=(cl == 0 and d == 0),
                            stop=(cl == CPC - 1 and d == dmax),
                            skip_group_check=True)

            emit_gt2_half(0)
            for cl in range(0, 4):
                emit_conv_cl(cl)
            for cl in range(4, 8):
                # spread the half-1 Gt2 matmuls between conv blocks to keep
                # the PE duty cycle high (a contiguous block re-throttles HAM)
                emit_gt2_half(1, jts=range(4 * (cl - 4), 4 * (cl - 3)))
                emit_conv_cl(cl)
            for cl in range(8, CPC):
                emit_conv_cl(cl)

            # ---------- epilogue: transpose (tloc, (beta,o)) -> (o, t) ----------
            # f32r operands: single-pass transpose at 1.5 cyc/row (vs 4 for
            # fp32) with ~1e-4 rounding, far below the conv dtype error
            F32R = mybir.dt.float32r
            identf = const.tile([128, 128], F32)
            make_identity(nc, identf[:])
            ident = const.tile([128, 128], F32R)
            nc.vector.tensor_copy(ident[:], identf[:])
            for w in range(2):
                out_sb = drn.tile([COUT, T // 2], F32, name=f"osb{w}", bufs=1)
                sb_d = drn.tile([128, 512], F32R)
                nc.vector.tensor_copy(sb_d[:], psw[w][:])
                for beta in range(8):
                    pt = pst.tile([COUT, 128], F32R)
                    nc.tensor.transpose(pt[:], sb_d[:, beta * 64:(beta + 1) * 64],
                                        ident[:])
                    dst = out_sb[:, beta * 128:(beta + 1) * 128]
                    if beta % 2 == 0:
                        nc.vector.tensor_copy(dst, pt[:])
                    else:
                        nc.scalar.copy(dst, pt[:])
                yv = y.ap().rearrange("o (w t) -> o w t", w=2)[:, w, :]
                nc.sync.dma_start(out=yv, in_=out_sb[:])

    nc.compile()
    return nc


def kernel(x, pos_rel, w1, b1, om1, w2, b2, om2, w3, b3, bias,
           dt_conv_name: str = "float32r"):
    from concourse.bass_utils import run_bass_kernel_spmd

    x = np.asarray(x, dtype=np.float32)
    pos_rel = np.asarray(pos_rel, dtype=np.float32)
    w1 = np.asarray(w1, dtype=np.float32)
    b1 = np.asarray(b1, dtype=np.float32)
    om1 = float(np.asarray(om1))
    w2 = np.asarray(w2, dtype=np.float32)
    b2 = np.asarray(b2, dtype=np.float32)
    om2 = float(np.asarray(om2))
    w3 = np.asarray(w3, dtype=np.float32)
    b3 = np.asarray(b3, dtype=np.float32)
    bias = np.asarray(bias, dtype=np.float32)

    # block-reversed positions (within each 128-tap tile), taps 0..2047 only,
    # replicated to DK partitions for the broadcast-free h1 compute
    posr_row = pos_rel[:T].reshape(T // 128, 128)[:, ::-1].reshape(T)
    posr = np.ascontiguousarray(
        np.broadcast_to(posr_row[None, :], (DK, T)), dtype=np.float32)

    w1v = (om1 * w1).reshape(DK, 1).astype(np.float32)
    b1v = (om1 * b1).reshape(DK, 1).astype(np.float32)
    w2t = np.ascontiguousarray(w2.T, dtype=np.float32)
    b2v = b2.reshape(DK, 1).astype(np.float32)  # om2 applied as ACT scale

    nc = _build_program(om2, dt_conv_name)

    # per-core inputs
    in_maps = []
    for core in range(N_CORES):
        b, h = divmod(core, 2)
        ci0 = h * CPC
        # w3aug[d, cl*64 + o] = w3[o*32 + ci0 + cl, d]; row DK = b3 slice
        w3_r = w3.reshape(COUT, CIN, DK)[:, ci0:ci0 + CPC, :]   # (o, cl, d)
        w3a = np.transpose(w3_r, (2, 1, 0)).reshape(DK, CPC * COUT)  # d,(cl,o)
        b3_r = b3.reshape(COUT, CIN)[:, ci0:ci0 + CPC]          # (o, cl)
        b3a = np.transpose(b3_r, (1, 0)).reshape(1, CPC * COUT)  # (cl, o)
        w3aug = np.concatenate([w3a, b3a], axis=0).astype(np.float32)
        in_maps.append({
            "xs": np.ascontiguousarray(x[b, ci0:ci0 + CPC, :]),
            "posr": posr,
            "w1v": w1v, "b1v": b1v, "w2t": w2t, "b2v": b2v,
            "w3aug": np.ascontiguousarray(w3aug),
        })

    res = run_bass_kernel_spmd(nc, in_maps, list(range(N_CORES)))

    out = np.empty((B, COUT, T), dtype=np.float32)
    for b in range(B):
        out[b] = res.results[2 * b]["y"] + res.results[2 * b + 1]["y"]
    out += bias[None, :, None]
    return out
